# revision 30
# baseline (speedup 1.0000x reference)
"""Trainium2 Bass kernel for the light-field disparity cost-volume build.

Input  x:   (2, 16, 25, 128, 128) f32  (b, c, n=angRes^2, h, w)
Output:     (2, 16, 25, 9, 128, 128) f32  (b, c, n, D, h, w)

out[b,c,(a1,a2),d,y,x] = x[b,c,(a1,a2), y + d*(2-a1), x + d*(2-a2)]
(zero outside the image), d in [-4, 4].

Pure data movement. Sharding: the 32 (b*c) slices split 4-per-core over
8 NeuronCores (data parallel, no cross-core communication).

Strategy (v2, big-descriptor): the baseline paid one 512B DMA
descriptor per shifted output row (~80k descriptors/core), which caps
the HWDGE rings at ~44 GB/s each. Here the column shift (the thing that
breaks DRAM-side row contiguity) is done by the DVE as an SBUF->SBUF
strided tensor_copy, so every output tile is materialized in SBUF with
32 consecutive output rows per partition. Stores then use 16 KB
descriptors and the row shift is absorbed into the store's DRAM offset
(any run of rows of a tile is contiguous in DRAM).

Layout (per partition free dim, f32 elems):
  [0 .. 5*4096)    RAW: a2-major columns. Column a2 (4096 elems) holds
                   views (a1, a2) for all a1, s on partitions
                   p = 16*a1 + 4*s + g, each partition = 32 consecutive
                   image rows (g-th row group) of that (view, slice).
  [RAW .. +2*4096) STAGING: 2 slots, same partition map as a RAW
                   column; slot holds the column-shifted copy for one
                   (d, a2) combo, margins zeroed.
  [ZOFF .. +1024)  zeros for the zero-row bands.

Work split:
  DVE     column-shift copies (u16-bitcast tensor_copy, 4x mode) +
          margin memsets, one combo (d, a2 != 2) at a time.
  sync    ring A: per combo, full-group + edge stores for a1 {0,1};
          d=0 tiles (straight copy from RAW); half the zero bands.
  scalar  ring B: per combo, stores for a1 {2,3,4}; other zero bands.
  gpsimd  loads (25 view DMAs), then a2==2 (c==0) tiles straight from
          RAW with row shift in the store offset.
"""

from contextlib import ExitStack

import numpy as np

import concourse.bass as bass
import concourse.mybir as mybir
from concourse.bass import AP
from concourse.bass_utils import run_bass_kernel_spmd

F32 = mybir.dt.float32
U16 = mybir.dt.uint16

B, C, NV, H, W = 2, 16, 25, 128, 128
A = 5
MIND, MAXD = -4, 4
D = MAXD - MIND + 1
NCORES = 8
NS = (B * C) // NCORES      # slices per core = 4

RPP = 32                    # image rows per partition
G = H // RPP                # row groups per tile = 4
FREE = RPP * W              # elems per partition per (view, slice) = 4096

X_V = H * W                 # input view stride (elems)
X_S = NV * X_V              # input slice stride
O_T = H * W                 # output tile stride
O_V = D * O_T               # output view stride
O_S = NV * O_V              # output slice stride

SOFF = A * FREE             # staging offset (after 5 RAW columns)
ZOFF = SOFF + 2 * FREE      # zeros region offset
ZLEN = 1024
PITCH = ZOFF + ZLEN

D_LIST = [d for d in range(MIND, MAXD + 1) if d != 0]
CORD = [0, 1, 3, 4]         # a2 columns with c != 0, in load order
COMBOS = [(a2, d) for a2 in CORD for d in D_LIST]   # 32 combos


def _p0(a1, g=0, s=0):
    """Partition index of (a1, g, s) within a column (g-major, s-minor).

    SBUF DMA APs must keep the partition dim as dim 0 with stride of one
    partition, so every DMA below addresses a DENSE partition range;
    iteration order over partitions is (g outer, s inner).
    """
    return 16 * a1 + 4 * g + s


def _build_nc():
    nc = bass.Bass()
    x = nc.dram_tensor("x", [NS, NV, H, W], F32, kind="ExternalInput")
    out = nc.dram_tensor("out", [NS, NV, D, H, W], F32, kind="ExternalOutput")

    # zero-band jobs (d, a1) with r != 0; batched over (a2, s) in one DMA
    zjobs = [
        (d, a1)
        for a1 in range(A)
        for d in D_LIST
        if d * (A // 2 - a1) != 0
    ]

    with (
        ExitStack() as stack,
        nc.sbuf_tensor([128, PITCH], F32) as buf,
        nc.semaphore("vsem") as vsem,     # staged combos (1/combo, DVE-ordered)
        nc.semaphore("zsem") as zsem,     # zeros region ready
        nc.semaphore("gsem") as gsem,     # gpsimd store completions
        nc.Block() as block,
    ):
        # Waits on a DMA-completion semaphore are only safe at its full
        # running total (each dma_start's 16 increments land unordered
        # across SDMA engines). Hence: one sem per a2 column for loads
        # (waited at 80 = all 5 view loads), and one sem per (ring,
        # staging slot) for combo stores (waited at 64/80 per past use).
        lsc = [stack.enter_context(nc.semaphore(f"lsc{j}")) for j in range(A)]
        fA = [stack.enter_context(nc.semaphore(f"fA{i}")) for i in range(2)]
        fB = [stack.enter_context(nc.semaphore(f"fB{i}")) for i in range(2)]
        pe = [stack.enter_context(nc.semaphore(f"pe{i}")) for i in range(2)]
        mA = stack.enter_context(nc.semaphore("mA"))  # sync misc stores
        mB = stack.enter_context(nc.semaphore("mB"))  # scalar misc stores

        @block.vector
        def _(vector):
            vector.memset(AP(buf, ZOFF, [[PITCH, 128], [1, ZLEN]]), 0.0)\
                .then_inc(zsem, 1)
            cur_col = None
            for k, (a2, d) in enumerate(COMBOS):
                c = d * (A // 2 - a2)
                if a2 != cur_col:
                    cur_col = a2
                    vector.wait_ge(lsc[a2], 80)
                if k >= 2:
                    # slot k%2 was last used by combo k-2; wait for its
                    # stores: 5 fulls (rings, a1=2 alternating) + 4
                    # edges (pool)
                    i = k % 2
                    vector.wait_ge(fA[i], (32 if i == 0 else 48) * (k // 2))
                    vector.wait_ge(fB[i], (48 if i == 0 else 32) * (k // 2))
                    vector.wait_ge(pe[i], 64 * (k // 2))
                so = SOFF + (k % 2) * FREE
                n = W - abs(c)
                src_off = a2 * FREE + max(c, 0)
                dst_off = so + max(-c, 0)
                # u16 bitcast: 2x elem counts/strides, 4x DVE mode
                vector.tensor_copy(
                    out=AP(buf, dst_off, [[PITCH, 80], [W, RPP], [1, n]]
                           ).bitcast(U16),
                    in_=AP(buf, src_off, [[PITCH, 80], [W, RPP], [1, n]]
                           ).bitcast(U16),
                )
                m_off = so + (W - c if c > 0 else 0)
                vector.memset(
                    AP(buf, m_off, [[PITCH, 80], [W, RPP], [1, abs(c)]]), 0.0
                ).then_inc(vsem, 1)
            nh = len(COMBOS) // 2
            for i in range(2):
                vector.wait_ge(fA[i], (32 if i == 0 else 48) * nh)
                vector.wait_ge(fB[i], (48 if i == 0 else 32) * nh)
                vector.wait_ge(pe[i], 64 * nh)

        def full_store(engine, a2, d, a1, sem, src_col_off, mdld=1024):
            """Full row-groups of tile (*, a1*5+a2, d) from SBUF.

            Partitions (g, s) for g in [g0, g0+ng) are dense; iteration
            is g outer, s inner on both sides. For HWDGE rings,
            max_dma_last_dim=1024 splits each partition's 16KB run into
            4KB descriptors so the HWDGE's chunk-of-4 engine dealing
            spreads the DMA over 12-16 SDMA engines instead of 3-4.
            SWDGE (gpsimd) round-robins whole descriptors over all 16
            engines, so it keeps 16KB descriptors (mdld=None).
            """
            r = d * (A // 2 - a1)
            g0 = 1 if r > 0 else 0
            ng = G if r == 0 else G - 1
            v = a1 * A + a2
            engine.dma_start(
                out=AP(out, v * O_V + (d - MIND) * O_T + (RPP * g0 - r) * W,
                       [[FREE, ng], [O_S, NS], [1, FREE]]),
                in_=AP(buf, _p0(a1, g0) * PITCH + src_col_off,
                       [[PITCH, 4 * ng], [1, FREE]]),
                max_dma_last_dim=mdld,
            ).then_inc(sem, 16)

        def edge_store(engine, a2, d, a1, sem, src_col_off, split=True):
            """Partial row-group at the shifted edge (r != 0 only)."""
            r = d * (A // 2 - a1)
            nr = RPP - abs(r)
            v = a1 * A + a2
            if r > 0:
                # group 0, input rows [r, 32) -> output rows [0, 32-r)
                src = _p0(a1, 0) * PITCH + src_col_off + r * W
                dst = v * O_V + (d - MIND) * O_T
            else:
                # group 3, input rows [96, 128-|r|) -> out [96+|r|, 128)
                src = _p0(a1, G - 1) * PITCH + src_col_off
                dst = v * O_V + (d - MIND) * O_T + (96 - r) * W
            # max_dma_last_dim is in BYTES: 1KB descriptors -> 48-62 per
            # DMA -> ~12-15 HWDGE chunks -> spread over most SDMA
            # engines. SWDGE (split=False): keep whole 12-15KB descs.
            engine.dma_start(
                out=AP(out, dst, [[O_S, NS], [1, nr * W]]),
                in_=AP(buf, src, [[PITCH, NS], [1, nr * W]]),
                max_dma_last_dim=1024 if split else None,
            ).then_inc(sem, 16)

        def zero_band(engine, d, a1, sem):
            r = d * (A // 2 - a1)
            dst = (a1 * A) * O_V + (d - MIND) * O_T + ((H - r) * W if r > 0 else 0)
            engine.dma_start(
                out=AP(out, dst, [[O_V, A], [O_S, NS], [1, abs(r) * W]]),
                in_=AP(buf, ZOFF, [[PITCH, A * NS], [1, abs(r) * W]]),
            ).then_inc(sem, 16)

        def d0_store(engine, a2, a1, sem):
            v = a1 * A + a2
            engine.dma_start(
                out=AP(out, v * O_V + (0 - MIND) * O_T,
                       [[FREE, G], [O_S, NS], [1, FREE]]),
                in_=AP(buf, _p0(a1) * PITCH + a2 * FREE,
                       [[PITCH, 4 * G], [1, FREE]]),
            ).then_inc(sem, 16)

        # Rings issue the full-group stores back-to-back: a sustained
        # descriptor stream keeps the HWDGE engine round-robin advancing
        # (it resets to engine 0 whenever the queue drains), spreading
        # the 16KB descriptors over all 16 SDMA engines. a1==2
        # alternates between the rings per combo to balance issue time.
        @block.sync
        def _(sync):
            cur_col = None
            az = [(d, a1) for d in D_LIST for a1 in range(A)][:20]
            nm = 0
            for k, (a2, d) in enumerate(COMBOS):
                if a2 != cur_col:
                    cur_col = a2
                    sync.wait_ge(lsc[a2], 80)
                sync.wait_ge(vsem, k + 1)
                so = SOFF + (k % 2) * FREE
                a1s = (0, 1, 2) if k % 2 else (0, 1)
                for a1 in a1s:
                    full_store(sync, a2, d, a1, fA[k % 2], so, mdld=None)
                # weave in the first half of the a2==2 fulls (from RAW)
                if k >= 12 and k - 12 < len(az):
                    if k == 12:
                        sync.wait_ge(lsc[2], 80)
                    dd, aa = az[k - 12]
                    full_store(sync, 2, dd, aa, mA, 2 * FREE, mdld=None)
                    nm += 1
            nh = len(COMBOS) // 2
            for i in range(2):
                sync.wait_ge(fA[i], (32 if i == 0 else 48) * nh)
            sync.wait_ge(mA, 16 * nm)

        @block.scalar
        def _(scalar):
            cur_col = None
            az = [(d, a1) for d in D_LIST for a1 in range(A)][20:]
            nm = 0
            for k, (a2, d) in enumerate(COMBOS):
                if a2 != cur_col:
                    cur_col = a2
                    scalar.wait_ge(lsc[a2], 80)
                scalar.wait_ge(vsem, k + 1)
                so = SOFF + (k % 2) * FREE
                a1s = (3, 4) if k % 2 else (2, 3, 4)
                for a1 in a1s:
                    full_store(scalar, a2, d, a1, fB[k % 2], so, mdld=None)
                if k >= 12 and k - 12 < len(az):
                    if k == 12:
                        scalar.wait_ge(lsc[2], 80)
                    dd, aa = az[k - 12]
                    full_store(scalar, 2, dd, aa, mB, 2 * FREE, mdld=None)
                    nm += 1
            nh = len(COMBOS) // 2
            for i in range(2):
                scalar.wait_ge(fB[i], (48 if i == 0 else 32) * nh)
            scalar.wait_ge(mB, 16 * nm)

        @block.gpsimd
        def _(gpsimd):
            # loads: one DMA per view, a2-column order CORD + [2]
            for a2 in CORD + [2]:
                for a1 in range(A):
                    gpsimd.dma_start(
                        out=AP(buf, _p0(a1) * PITCH + a2 * FREE,
                               [[PITCH, 16], [1, FREE]]),
                        in_=AP(x, (a1 * A + a2) * X_V,
                               [[FREE, G], [X_S, NS], [1, FREE]]),
                    ).then_inc(lsc[a2], 16)
            n_misc = 0
            # edges per combo + d0 (from RAW, at column starts) + a2==2
            # edges + zero bands, all on SWDGE (whole 12-16KB
            # descriptors, round-robined over all 16 SDMA engines)
            cur_col = None
            a2e = [(d, a1) for d in D_LIST for a1 in range(A)
                   if d * (A // 2 - a1) != 0]
            ncols = 0
            for k, (a2, d) in enumerate(COMBOS):
                if a2 != cur_col:
                    cur_col = a2
                    gpsimd.wait_ge(lsc[a2], 80)
                    for a1 in range(A):
                        d0_store(gpsimd, a2, a1, gsem)
                        n_misc += 1
                gpsimd.wait_ge(vsem, k + 1)
                so = SOFF + (k % 2) * FREE
                for a1 in (0, 1, 3, 4):
                    edge_store(gpsimd, a2, d, a1, pe[k % 2], so, split=False)
                if k >= 12 and k - 12 < len(a2e):
                    if k == 12:
                        gpsimd.wait_ge(lsc[2], 80)
                        for a1 in range(A):
                            d0_store(gpsimd, 2, a1, gsem)
                            n_misc += 1
                    dd, aa = a2e[k - 12]
                    edge_store(gpsimd, 2, dd, aa, gsem, 2 * FREE, split=False)
                    n_misc += 1
                    if k - 12 + 20 < len(a2e):
                        dd, aa = a2e[k - 12 + 20]
                        edge_store(gpsimd, 2, dd, aa, gsem, 2 * FREE,
                                   split=False)
                        n_misc += 1
                if k == 0:
                    gpsimd.wait_ge(zsem, 1)
                if k < len(zjobs):
                    dz, az_ = zjobs[k]
                    zero_band(gpsimd, dz, az_, gsem)
                    n_misc += 1
            nh = len(COMBOS) // 2
            for i in range(2):
                gpsimd.wait_ge(pe[i], 64 * nh)
            gpsimd.wait_ge(gsem, 16 * n_misc)

    return nc


_NC = None


def _get_nc():
    global _NC
    if _NC is None:
        _NC = _build_nc()
    return _NC


def kernel(x: np.ndarray) -> np.ndarray:
    assert x.shape == (B, C, NV, H, W), x.shape
    xs = np.ascontiguousarray(x.astype(np.float32, copy=False)).reshape(
        B * C, NV, H, W
    )
    in_maps = [{"x": xs[NS * k : NS * (k + 1)]} for k in range(NCORES)]
    res = run_bass_kernel_spmd(_get_nc(), in_maps, core_ids=list(range(NCORES)))
    out = np.concatenate([r["out"] for r in res.results], axis=0)
    return out.reshape(B, C, NV, D, H, W)


# revision 33
# speedup vs baseline: 1.7372x; 1.7372x over previous
"""Trainium2 Bass kernel for the light-field disparity cost-volume build.

Input  x:   (2, 16, 25, 128, 128) f32  (b, c, n=angRes^2, h, w)
Output:     (2, 16, 25, 9, 128, 128) f32  (b, c, n, D, h, w)

out[b,c,(a1,a2),d,y,x] = x[b,c,(a1,a2), y + d*(2-a1), x + d*(2-a2)]
(zero outside the image), d in [-4, 4].

Pure data movement. Sharding: the 32 (b*c) slices split 4-per-core over
8 NeuronCores (data parallel, no cross-core communication).

Strategy (big-descriptor): the baseline paid one 512B DMA descriptor
per shifted output row (~80k descriptors/core), which caps the HWDGE
rings at ~44 GB/s each. Here the column shift (the thing that breaks
DRAM-side row contiguity) is done by the DVE as an SBUF->SBUF strided
tensor_copy, so every output tile is materialized in SBUF with 32
consecutive output rows per partition; stores then use 16 KB
descriptors and the row shift is absorbed into the store's DRAM offset
(any run of rows of a tile is contiguous in DRAM). 540us -> 382us.

Measured HW behavior that shaped the work split below:
  - HWDGE (sync/scalar rings): each dma_start's descriptors are dealt
    to SDMA engines in chunks of ~4 (16KB descs; ~16 for 1KB descs),
    restarting at engine 0 for EVERY DMA -> a <=16-descriptor ring DMA
    only ever uses engines 0-3. Ring issue cost ~0.6us + ~13ns/desc.
  - SWDGE (gpsimd): descriptors round-robin continuously over all 16
    SDMA engines regardless of DMA size, ~0.7us/dma_start on the Q7.
  - Hence: the bulk (full 4-partition-group stores, 12-16 x 16KB descs
    each) goes on gpsimd/SWDGE; the rings carry only the small edge
    stores (split into 1KB descriptors) and zero bands.
  - max_dma_last_dim is in BYTES.

Layout (per partition free dim, f32 elems):
  [0 .. 5*4096)    RAW: a2-major columns. Column a2 (4096 elems) holds
                   views (a1, a2) for all a1, s on partitions
                   p = 16*a1 + 4*g + s, each partition = 32 consecutive
                   image rows (g-th row group) of that (view, slice).
  [RAW .. +2*4096) STAGING: 2 slots, same partition map as a RAW
                   column; slot holds the column-shifted copy for one
                   (d, a2) combo, margins zeroed.
  [ZOFF .. +1024)  zeros for the zero-row bands.

Work split:
  DVE     column-shift copies (u16-bitcast tensor_copy, 4x mode) +
          margin memsets, one combo (d != 0, a2 != 2) at a time.
  gpsimd  loads (25 view DMAs); d=0 tiles (one DRAM->DRAM copy); per
          combo the 5 full-group stores from staging; a2==2 (c==0)
          full-group stores straight from RAW (row shift in the store
          offset).
  sync    per combo, edge stores (the partial row group at the shifted
          boundary) for a1 {0,1}; half the a2==2 edges + zero bands.
  scalar  same for a1 {3,4} and the other half.
"""

from contextlib import ExitStack

import numpy as np

import concourse.bass as bass
import concourse.mybir as mybir
from concourse.bass import AP
from concourse.bass_utils import run_bass_kernel_spmd

F32 = mybir.dt.float32
U16 = mybir.dt.uint16

B, C, NV, H, W = 2, 16, 25, 128, 128
A = 5
MIND, MAXD = -4, 4
D = MAXD - MIND + 1
NCORES = 8
NS = (B * C) // NCORES      # slices per core = 4

RPP = 32                    # image rows per partition
G = H // RPP                # row groups per tile = 4
FREE = RPP * W              # elems per partition per (view, slice) = 4096

X_V = H * W                 # input view stride (elems)
X_S = NV * X_V              # input slice stride
O_T = H * W                 # output tile stride
O_V = D * O_T               # output view stride
O_S = NV * O_V              # output slice stride

SOFF = A * FREE             # staging offset (after 5 RAW columns)
ZOFF = SOFF + 2 * FREE      # zeros region offset
ZLEN = 1024
PITCH = ZOFF + ZLEN

D_LIST = [d for d in range(MIND, MAXD + 1) if d != 0]
CORD = [0, 1, 3, 4]         # a2 columns with c != 0, in load order
COMBOS = [(a2, d) for a2 in CORD for d in D_LIST]   # 32 combos


def _p0(a1, g=0, s=0):
    """Partition index of (a1, g, s) within a column (g-major, s-minor).

    SBUF DMA APs must keep the partition dim as dim 0 with stride of one
    partition, so every DMA below addresses a DENSE partition range;
    iteration order over partitions is (g outer, s inner).
    """
    return 16 * a1 + 4 * g + s


def _build_nc():
    nc = bass.Bass()
    x = nc.dram_tensor("x", [NS, NV, H, W], F32, kind="ExternalInput")
    out = nc.dram_tensor("out", [NS, NV, D, H, W], F32, kind="ExternalOutput")

    # zero-band jobs (d, a1) with r != 0; batched over (a2, s) in one DMA
    zjobs = [
        (d, a1)
        for a1 in range(A)
        for d in D_LIST
        if d * (A // 2 - a1) != 0
    ]

    with (
        ExitStack() as stack,
        nc.sbuf_tensor([128, PITCH], F32) as buf,
        nc.semaphore("vsem") as vsem,     # staged combos (1/combo, DVE-ordered)
        nc.semaphore("zsem") as zsem,     # zeros region ready
        nc.semaphore("gsem") as gsem,     # gpsimd store completions
        nc.Block() as block,
    ):
        # Waits on a DMA-completion semaphore are only safe at its full
        # running total (each dma_start's 16 increments land unordered
        # across SDMA engines). Hence: one sem per a2 column for loads
        # (waited at 80 = all 5 view loads), and one sem per (ring,
        # staging slot) for combo stores (waited at 64/80 per past use).
        lsc = [stack.enter_context(nc.semaphore(f"lsc{j}")) for j in range(A)]
        gf = [stack.enter_context(nc.semaphore(f"gf{i}")) for i in range(2)]
        eA = [stack.enter_context(nc.semaphore(f"eA{i}")) for i in range(2)]
        eB = [stack.enter_context(nc.semaphore(f"eB{i}")) for i in range(2)]
        mA = stack.enter_context(nc.semaphore("mA"))  # sync misc stores
        mB = stack.enter_context(nc.semaphore("mB"))  # scalar misc stores

        @block.vector
        def _(vector):
            vector.memset(AP(buf, ZOFF, [[PITCH, 128], [1, ZLEN]]), 0.0)\
                .then_inc(zsem, 1)
            cur_col = None
            for k, (a2, d) in enumerate(COMBOS):
                c = d * (A // 2 - a2)
                if a2 != cur_col:
                    cur_col = a2
                    vector.wait_ge(lsc[a2], 80)
                if k >= 2:
                    # slot k%2 was last used by combo k-2; wait for its
                    # stores: 5 fulls (pool) + 2+2 edges (rings)
                    vector.wait_ge(gf[k % 2], 80 * (k // 2))
                    vector.wait_ge(eA[k % 2], 32 * (k // 2))
                    vector.wait_ge(eB[k % 2], 32 * (k // 2))
                so = SOFF + (k % 2) * FREE
                n = W - abs(c)
                src_off = a2 * FREE + max(c, 0)
                dst_off = so + max(-c, 0)
                # u16 bitcast: 2x elem counts/strides, 4x DVE mode
                vector.tensor_copy(
                    out=AP(buf, dst_off, [[PITCH, 80], [W, RPP], [1, n]]
                           ).bitcast(U16),
                    in_=AP(buf, src_off, [[PITCH, 80], [W, RPP], [1, n]]
                           ).bitcast(U16),
                )
                m_off = so + (W - c if c > 0 else 0)
                vector.memset(
                    AP(buf, m_off, [[PITCH, 80], [W, RPP], [1, abs(c)]]), 0.0
                ).then_inc(vsem, 1)
            for i in range(2):
                vector.wait_ge(gf[i], 80 * (len(COMBOS) // 2))
                vector.wait_ge(eA[i], 32 * (len(COMBOS) // 2))
                vector.wait_ge(eB[i], 32 * (len(COMBOS) // 2))

        def full_store(engine, a2, d, a1, sem, src_col_off, mdld=1024):
            """Full row-groups of tile (*, a1*5+a2, d) from SBUF.

            Partitions (g, s) for g in [g0, g0+ng) are dense; iteration
            is g outer, s inner on both sides. For HWDGE rings,
            max_dma_last_dim=1024 splits each partition's 16KB run into
            4KB descriptors so the HWDGE's chunk-of-4 engine dealing
            spreads the DMA over 12-16 SDMA engines instead of 3-4.
            SWDGE (gpsimd) round-robins whole descriptors over all 16
            engines, so it keeps 16KB descriptors (mdld=None).
            """
            r = d * (A // 2 - a1)
            g0 = 1 if r > 0 else 0
            ng = G if r == 0 else G - 1
            v = a1 * A + a2
            engine.dma_start(
                out=AP(out, v * O_V + (d - MIND) * O_T + (RPP * g0 - r) * W,
                       [[FREE, ng], [O_S, NS], [1, FREE]]),
                in_=AP(buf, _p0(a1, g0) * PITCH + src_col_off,
                       [[PITCH, 4 * ng], [1, FREE]]),
                max_dma_last_dim=mdld,
            ).then_inc(sem, 16)

        def edge_store(engine, a2, d, a1, sem, src_col_off, split=True):
            """Partial row-group at the shifted edge (r != 0 only)."""
            r = d * (A // 2 - a1)
            nr = RPP - abs(r)
            v = a1 * A + a2
            if r > 0:
                # group 0, input rows [r, 32) -> output rows [0, 32-r)
                src = _p0(a1, 0) * PITCH + src_col_off + r * W
                dst = v * O_V + (d - MIND) * O_T
            else:
                # group 3, input rows [96, 128-|r|) -> out [96+|r|, 128)
                src = _p0(a1, G - 1) * PITCH + src_col_off
                dst = v * O_V + (d - MIND) * O_T + (96 - r) * W
            # max_dma_last_dim is in BYTES: 1KB descriptors -> 48-62 per
            # DMA -> ~12-15 HWDGE chunks -> spread over most SDMA
            # engines. SWDGE (split=False): keep whole 12-15KB descs.
            engine.dma_start(
                out=AP(out, dst, [[O_S, NS], [1, nr * W]]),
                in_=AP(buf, src, [[PITCH, NS], [1, nr * W]]),
                max_dma_last_dim=1024 if split else None,
            ).then_inc(sem, 16)

        def zero_band(engine, d, a1, sem):
            r = d * (A // 2 - a1)
            dst = (a1 * A) * O_V + (d - MIND) * O_T + ((H - r) * W if r > 0 else 0)
            engine.dma_start(
                out=AP(out, dst, [[O_V, A], [O_S, NS], [1, abs(r) * W]]),
                in_=AP(buf, ZOFF, [[PITCH, A * NS], [1, abs(r) * W]]),
            ).then_inc(sem, 16)

        @block.sync
        def _(sync):
            # edges a1 {0,1} per combo + half the a2==2 edges and zero
            # bands, 1KB descriptors (best observed HWDGE engine spread)
            cur_col = None
            az = [(d, a1) for d in D_LIST for a1 in (0, 1)]
            zA = zjobs[0::2]
            nm = 0
            sync.wait_ge(zsem, 1)
            for k, (a2, d) in enumerate(COMBOS):
                if a2 != cur_col:
                    cur_col = a2
                    sync.wait_ge(lsc[a2], 80)
                sync.wait_ge(vsem, k + 1)
                so = SOFF + (k % 2) * FREE
                for a1 in (0, 1):
                    edge_store(sync, a2, d, a1, eA[k % 2], so)
                if k >= 16 and k - 16 < len(az):
                    if k == 16:
                        sync.wait_ge(lsc[2], 80)
                    dd, aa = az[k - 16]
                    edge_store(sync, 2, dd, aa, mA, 2 * FREE)
                    nm += 1
                if k < len(zA):
                    dz, az_ = zA[k]
                    zero_band(sync, dz, az_, mA)
                    nm += 1
            for i in range(2):
                sync.wait_ge(eA[i], 32 * (len(COMBOS) // 2))
            sync.wait_ge(mA, 16 * nm)

        @block.scalar
        def _(scalar):
            cur_col = None
            az = [(d, a1) for d in D_LIST for a1 in (3, 4)]
            zB = zjobs[1::2]
            nm = 0
            scalar.wait_ge(zsem, 1)
            for k, (a2, d) in enumerate(COMBOS):
                if a2 != cur_col:
                    cur_col = a2
                    scalar.wait_ge(lsc[a2], 80)
                scalar.wait_ge(vsem, k + 1)
                so = SOFF + (k % 2) * FREE
                for a1 in (3, 4):
                    edge_store(scalar, a2, d, a1, eB[k % 2], so)
                if k >= 16 and k - 16 < len(az):
                    if k == 16:
                        scalar.wait_ge(lsc[2], 80)
                    dd, aa = az[k - 16]
                    edge_store(scalar, 2, dd, aa, mB, 2 * FREE)
                    nm += 1
                if k < len(zB):
                    dz, az_ = zB[k]
                    zero_band(scalar, dz, az_, mB)
                    nm += 1
            for i in range(2):
                scalar.wait_ge(eB[i], 32 * (len(COMBOS) // 2))
            scalar.wait_ge(mB, 16 * nm)

        @block.gpsimd
        def _(gpsimd):
            # loads: one DMA per view, a2-column order CORD + [2]
            for a2 in CORD + [2]:
                for a1 in range(A):
                    gpsimd.dma_start(
                        out=AP(buf, _p0(a1) * PITCH + a2 * FREE,
                               [[PITCH, 16], [1, FREE]]),
                        in_=AP(x, (a1 * A + a2) * X_V,
                               [[FREE, G], [X_S, NS], [1, FREE]]),
                    ).then_inc(lsc[a2], 16)
            # d=0 tiles: straight DRAM->DRAM copy of every view (one DMA,
            # big descriptors, SWDGE spreads them over all 16 engines)
            gpsimd.dma_start(
                out=AP(out, (0 - MIND) * O_T, [[O_V, NS * NV], [1, X_V]]),
                in_=AP(x, 0, [[X_V, NS * NV], [1, X_V]]),
                max_dma_last_dim=8192,
            ).then_inc(gsem, 16)
            n_misc = 1
            # full-group stores for every combo (SWDGE: 16KB descriptors,
            # round-robined over all 16 SDMA engines)
            cur_col = None
            a22 = [(d, a1) for d in D_LIST for a1 in range(A)]
            for k, (a2, d) in enumerate(COMBOS):
                if a2 != cur_col:
                    cur_col = a2
                    gpsimd.wait_ge(lsc[a2], 80)
                gpsimd.wait_ge(vsem, k + 1)
                so = SOFF + (k % 2) * FREE
                for a1 in range(A):
                    full_store(gpsimd, a2, d, a1, gf[k % 2], so, mdld=None)
                # weave in a2==2 fulls (c==0, straight from RAW) so they
                # overlap the combo pipeline
                if k >= 12 and k - 12 < len(a22):
                    if k == 12:
                        gpsimd.wait_ge(lsc[2], 80)
                    dd, aa = a22[k - 12]
                    full_store(gpsimd, 2, dd, aa, gsem, 2 * FREE, mdld=None)
                    n_misc += 1
                    dd, aa = a22[k - 12 + 20]
                    full_store(gpsimd, 2, dd, aa, gsem, 2 * FREE, mdld=None)
                    n_misc += 1
            for i in range(2):
                gpsimd.wait_ge(gf[i], 80 * (len(COMBOS) // 2))
            gpsimd.wait_ge(gsem, 16 * n_misc)

    return nc


_NC = None


def _get_nc():
    global _NC
    if _NC is None:
        _NC = _build_nc()
    return _NC


def kernel(x: np.ndarray) -> np.ndarray:
    assert x.shape == (B, C, NV, H, W), x.shape
    xs = np.ascontiguousarray(x.astype(np.float32, copy=False)).reshape(
        B * C, NV, H, W
    )
    in_maps = [{"x": xs[NS * k : NS * (k + 1)]} for k in range(NCORES)]
    res = run_bass_kernel_spmd(_get_nc(), in_maps, core_ids=list(range(NCORES)))
    out = np.concatenate([r["out"] for r in res.results], axis=0)
    return out.reshape(B, C, NV, D, H, W)


# revision 35
# speedup vs baseline: 1.8030x; 1.0379x over previous
"""Trainium2 Bass kernel for the light-field disparity cost-volume build.

Input  x:   (2, 16, 25, 128, 128) f32  (b, c, n=angRes^2, h, w)
Output:     (2, 16, 25, 9, 128, 128) f32  (b, c, n, D, h, w)

out[b,c,(a1,a2),d,y,x] = x[b,c,(a1,a2), y + d*(2-a1), x + d*(2-a2)]
(zero outside the image), d in [-4, 4].

Pure data movement. Sharding: the 32 (b*c) slices split 4-per-core over
8 NeuronCores (data parallel, no cross-core communication).

Strategy (big-descriptor): the baseline paid one 512B DMA descriptor
per shifted output row (~80k descriptors/core), which caps the HWDGE
rings at ~44 GB/s each. Here the column shift (the thing that breaks
DRAM-side row contiguity) is done by the DVE as an SBUF->SBUF strided
tensor_copy, so every output tile is materialized in SBUF with 32
consecutive output rows per partition; stores then use 16 KB
descriptors and the row shift is absorbed into the store's DRAM offset
(any run of rows of a tile is contiguous in DRAM). 540us -> 382us.

Measured HW behavior that shaped the work split below:
  - HWDGE (sync/scalar rings): each dma_start's descriptors are dealt
    to SDMA engines in chunks of ~4 (16KB descs; ~16 for 1KB descs),
    restarting at engine 0 for EVERY DMA -> a <=16-descriptor ring DMA
    only ever uses engines 0-3. Ring issue cost ~0.6us + ~13ns/desc.
  - SWDGE (gpsimd): descriptors round-robin continuously over all 16
    SDMA engines regardless of DMA size, ~0.7us/dma_start on the Q7.
  - Hence: the bulk (full 4-partition-group stores, 12-16 x 16KB descs
    each) goes on gpsimd/SWDGE; the rings carry only the small edge
    stores (split into 1KB descriptors) and zero bands.
  - max_dma_last_dim is in BYTES.

Layout (per partition free dim, f32 elems):
  [0 .. 5*4096)    RAW: a2-major columns. Column a2 (4096 elems) holds
                   views (a1, a2) for all a1, s on partitions
                   p = 16*a1 + 4*g + s, each partition = 32 consecutive
                   image rows (g-th row group) of that (view, slice).
  [RAW .. +2*4096) STAGING: 2 slots, same partition map as a RAW
                   column; slot holds the column-shifted copy for one
                   (d, a2) combo, margins zeroed.
  [ZOFF .. +1024)  zeros for the zero-row bands.

Work split:
  DVE     column-shift copies (u16-bitcast tensor_copy, 4x mode) +
          margin memsets, one combo (d != 0, a2 != 2) at a time.
  gpsimd  loads (25 view DMAs); d=0 tiles (one DRAM->DRAM copy); per
          combo the 5 full-group stores from staging; a2==2 (c==0)
          full-group stores straight from RAW (row shift in the store
          offset).
  sync    per combo, edge stores (the partial row group at the shifted
          boundary) for a1 {0,1}; half the a2==2 edges + zero bands.
  scalar  same for a1 {3,4} and the other half.
"""

from contextlib import ExitStack

import numpy as np

import concourse.bass as bass
import concourse.mybir as mybir
from concourse.bass import AP
from concourse.bass_utils import run_bass_kernel_spmd

F32 = mybir.dt.float32
U16 = mybir.dt.uint16

B, C, NV, H, W = 2, 16, 25, 128, 128
A = 5
MIND, MAXD = -4, 4
D = MAXD - MIND + 1
NCORES = 8
NS = (B * C) // NCORES      # slices per core = 4

RPP = 32                    # image rows per partition
G = H // RPP                # row groups per tile = 4
FREE = RPP * W              # elems per partition per (view, slice) = 4096

X_V = H * W                 # input view stride (elems)
X_S = NV * X_V              # input slice stride
O_T = H * W                 # output tile stride
O_V = D * O_T               # output view stride
O_S = NV * O_V              # output slice stride

SOFF = A * FREE             # staging offset (after 5 RAW columns)
NSLOT = 4                   # staging slots (pipeline depth)
ZOFF = SOFF + NSLOT * FREE  # zeros region offset
ZLEN = 1024
PITCH = ZOFF + ZLEN

D_LIST = [d for d in range(MIND, MAXD + 1) if d != 0]
CORD = [0, 1, 3, 4]         # a2 columns with c != 0, in load order
COMBOS = [(a2, d) for a2 in CORD for d in D_LIST]   # 32 combos


def _p0(a1, g=0, s=0):
    """Partition index of (a1, g, s) within a column (g-major, s-minor).

    SBUF DMA APs must keep the partition dim as dim 0 with stride of one
    partition, so every DMA below addresses a DENSE partition range;
    iteration order over partitions is (g outer, s inner).
    """
    return 16 * a1 + 4 * g + s


def _build_nc():
    nc = bass.Bass()
    x = nc.dram_tensor("x", [NS, NV, H, W], F32, kind="ExternalInput")
    out = nc.dram_tensor("out", [NS, NV, D, H, W], F32, kind="ExternalOutput")

    # zero-band jobs (d, a1) with r != 0; batched over (a2, s) in one DMA
    zjobs = [
        (d, a1)
        for a1 in range(A)
        for d in D_LIST
        if d * (A // 2 - a1) != 0
    ]

    with (
        ExitStack() as stack,
        nc.sbuf_tensor([128, PITCH], F32) as buf,
        nc.semaphore("vsem") as vsem,     # staged combos (1/combo, DVE-ordered)
        nc.semaphore("zsem") as zsem,     # zeros region ready
        nc.semaphore("gsem") as gsem,     # gpsimd store completions
        nc.Block() as block,
    ):
        # Waits on a DMA-completion semaphore are only safe at its full
        # running total (each dma_start's 16 increments land unordered
        # across SDMA engines). Hence: one sem per a2 column for loads
        # (waited at 80 = all 5 view loads), and one sem per (ring,
        # staging slot) for combo stores (waited at 64/80 per past use).
        lsc = [stack.enter_context(nc.semaphore(f"lsc{j}")) for j in range(A)]
        gf = [stack.enter_context(nc.semaphore(f"gf{i}")) for i in range(NSLOT)]
        eA = [stack.enter_context(nc.semaphore(f"eA{i}")) for i in range(NSLOT)]
        eB = [stack.enter_context(nc.semaphore(f"eB{i}")) for i in range(NSLOT)]

        @block.vector
        def _(vector):
            vector.memset(AP(buf, ZOFF, [[PITCH, 128], [1, ZLEN]]), 0.0)\
                .then_inc(zsem, 1)
            cur_col = None
            for k, (a2, d) in enumerate(COMBOS):
                c = d * (A // 2 - a2)
                if a2 != cur_col:
                    cur_col = a2
                    vector.wait_ge(lsc[a2], 80)
                if k >= NSLOT:
                    # slot k%NSLOT was last used by combo k-NSLOT; wait
                    # for its stores: 5 fulls (pool) + 2+2 edges (rings)
                    vector.wait_ge(gf[k % NSLOT], 80 * (k // NSLOT))
                    vector.wait_ge(eA[k % NSLOT], 32 * (k // NSLOT))
                    vector.wait_ge(eB[k % NSLOT], 32 * (k // NSLOT))
                so = SOFF + (k % NSLOT) * FREE
                n = W - abs(c)
                src_off = a2 * FREE + max(c, 0)
                dst_off = so + max(-c, 0)
                # u16 bitcast: 2x elem counts/strides, 4x DVE mode
                vector.tensor_copy(
                    out=AP(buf, dst_off, [[PITCH, 80], [W, RPP], [1, n]]
                           ).bitcast(U16),
                    in_=AP(buf, src_off, [[PITCH, 80], [W, RPP], [1, n]]
                           ).bitcast(U16),
                )
                m_off = so + (W - c if c > 0 else 0)
                vector.memset(
                    AP(buf, m_off, [[PITCH, 80], [W, RPP], [1, abs(c)]]), 0.0
                ).then_inc(vsem, 1)
            for i in range(NSLOT):
                vector.wait_ge(gf[i], 80 * (len(COMBOS) // NSLOT))
                vector.wait_ge(eA[i], 32 * (len(COMBOS) // NSLOT))
                vector.wait_ge(eB[i], 32 * (len(COMBOS) // NSLOT))

        def full_store(engine, a2, d, a1, sem, src_col_off, mdld=1024):
            """Full row-groups of tile (*, a1*5+a2, d) from SBUF.

            Partitions (g, s) for g in [g0, g0+ng) are dense; iteration
            is g outer, s inner on both sides. For HWDGE rings,
            max_dma_last_dim=1024 splits each partition's 16KB run into
            4KB descriptors so the HWDGE's chunk-of-4 engine dealing
            spreads the DMA over 12-16 SDMA engines instead of 3-4.
            SWDGE (gpsimd) round-robins whole descriptors over all 16
            engines, so it keeps 16KB descriptors (mdld=None).
            """
            r = d * (A // 2 - a1)
            g0 = 1 if r > 0 else 0
            ng = G if r == 0 else G - 1
            v = a1 * A + a2
            engine.dma_start(
                out=AP(out, v * O_V + (d - MIND) * O_T + (RPP * g0 - r) * W,
                       [[FREE, ng], [O_S, NS], [1, FREE]]),
                in_=AP(buf, _p0(a1, g0) * PITCH + src_col_off,
                       [[PITCH, 4 * ng], [1, FREE]]),
                max_dma_last_dim=mdld,
            ).then_inc(sem, 16)

        def edge_store(engine, a2, d, a1, sem, src_col_off, split=True):
            """Partial row-group at the shifted edge (r != 0 only)."""
            r = d * (A // 2 - a1)
            nr = RPP - abs(r)
            v = a1 * A + a2
            if r > 0:
                # group 0, input rows [r, 32) -> output rows [0, 32-r)
                src = _p0(a1, 0) * PITCH + src_col_off + r * W
                dst = v * O_V + (d - MIND) * O_T
            else:
                # group 3, input rows [96, 128-|r|) -> out [96+|r|, 128)
                src = _p0(a1, G - 1) * PITCH + src_col_off
                dst = v * O_V + (d - MIND) * O_T + (96 - r) * W
            # max_dma_last_dim is in BYTES: 1KB descriptors -> 48-62 per
            # DMA -> ~12-15 HWDGE chunks -> spread over most SDMA
            # engines. SWDGE (split=False): keep whole 12-15KB descs.
            engine.dma_start(
                out=AP(out, dst, [[O_S, NS], [1, nr * W]]),
                in_=AP(buf, src, [[PITCH, NS], [1, nr * W]]),
                max_dma_last_dim=1024 if split else None,
            ).then_inc(sem, 16)

        def zero_band(engine, d, a1, sem):
            r = d * (A // 2 - a1)
            dst = (a1 * A) * O_V + (d - MIND) * O_T + ((H - r) * W if r > 0 else 0)
            engine.dma_start(
                out=AP(out, dst, [[O_V, A], [O_S, NS], [1, abs(r) * W]]),
                in_=AP(buf, ZOFF, [[PITCH, A * NS], [1, abs(r) * W]]),
            ).then_inc(sem, 16)

        @block.sync
        def _(sync):
            # interior edge stores a1 {0,1} per combo (1KB descriptors)
            cur_col = None
            for k, (a2, d) in enumerate(COMBOS):
                if a2 != cur_col:
                    cur_col = a2
                    sync.wait_ge(lsc[a2], 80)
                sync.wait_ge(vsem, k + 1)
                so = SOFF + (k % NSLOT) * FREE
                for a1 in (0, 1):
                    edge_store(sync, a2, d, a1, eA[k % NSLOT], so)
            for i in range(NSLOT):
                sync.wait_ge(eA[i], 32 * (len(COMBOS) // NSLOT))

        @block.scalar
        def _(scalar):
            cur_col = None
            for k, (a2, d) in enumerate(COMBOS):
                if a2 != cur_col:
                    cur_col = a2
                    scalar.wait_ge(lsc[a2], 80)
                scalar.wait_ge(vsem, k + 1)
                so = SOFF + (k % NSLOT) * FREE
                for a1 in (3, 4):
                    edge_store(scalar, a2, d, a1, eB[k % NSLOT], so)
            for i in range(NSLOT):
                scalar.wait_ge(eB[i], 32 * (len(COMBOS) // NSLOT))

        @block.gpsimd
        def _(gpsimd):
            # loads: one DMA per view, a2-column order CORD + [2]
            for a2 in CORD + [2]:
                for a1 in range(A):
                    gpsimd.dma_start(
                        out=AP(buf, _p0(a1) * PITCH + a2 * FREE,
                               [[PITCH, 16], [1, FREE]]),
                        in_=AP(x, (a1 * A + a2) * X_V,
                               [[FREE, G], [X_S, NS], [1, FREE]]),
                    ).then_inc(lsc[a2], 16)
            # d=0 tiles: straight DRAM->DRAM copy of every view (one DMA,
            # big descriptors, SWDGE spreads them over all 16 engines)
            gpsimd.dma_start(
                out=AP(out, (0 - MIND) * O_T, [[O_V, NS * NV], [1, X_V]]),
                in_=AP(x, 0, [[X_V, NS * NV], [1, X_V]]),
                max_dma_last_dim=8192,
            ).then_inc(gsem, 16)
            n_misc = 1
            # per combo: the 5 full-group stores (16KB descriptors,
            # round-robined over all 16 SDMA engines), weaving in the
            # a2==2 fulls+edges (c==0, straight from RAW) and the zero
            # bands so everything overlaps the combo pipeline
            cur_col = None
            a22 = [(d, a1) for d in D_LIST for a1 in range(A)]
            a2e = [(d, a1) for d in D_LIST for a1 in range(A)
                   if d * (A // 2 - a1) != 0]
            for k, (a2, d) in enumerate(COMBOS):
                if a2 != cur_col:
                    cur_col = a2
                    gpsimd.wait_ge(lsc[a2], 80)
                gpsimd.wait_ge(vsem, k + 1)
                so = SOFF + (k % NSLOT) * FREE
                for a1 in range(A):
                    full_store(gpsimd, a2, d, a1, gf[k % NSLOT], so,
                               mdld=None)
                if k >= 12 and k - 12 < len(a22):
                    if k == 12:
                        gpsimd.wait_ge(lsc[2], 80)
                    dd, aa = a22[k - 12]
                    full_store(gpsimd, 2, dd, aa, gsem, 2 * FREE, mdld=None)
                    n_misc += 1
                    dd, aa = a22[k - 12 + 20]
                    full_store(gpsimd, 2, dd, aa, gsem, 2 * FREE, mdld=None)
                    n_misc += 1
                if k >= 12 and k - 12 < len(a2e):
                    dd, aa = a2e[k - 12]
                    edge_store(gpsimd, 2, dd, aa, gsem, 2 * FREE,
                               split=False)
                    n_misc += 1
                    if k - 12 + 20 < len(a2e):
                        dd, aa = a2e[k - 12 + 20]
                        edge_store(gpsimd, 2, dd, aa, gsem, 2 * FREE,
                                   split=False)
                        n_misc += 1
                if k == 0:
                    gpsimd.wait_ge(zsem, 1)
                if k < len(zjobs):
                    dz, az_ = zjobs[k]
                    zero_band(gpsimd, dz, az_, gsem)
                    n_misc += 1
            for i in range(NSLOT):
                gpsimd.wait_ge(gf[i], 80 * (len(COMBOS) // NSLOT))
            gpsimd.wait_ge(gsem, 16 * n_misc)

    return nc


_NC = None


def _get_nc():
    global _NC
    if _NC is None:
        _NC = _build_nc()
    return _NC


def kernel(x: np.ndarray) -> np.ndarray:
    assert x.shape == (B, C, NV, H, W), x.shape
    xs = np.ascontiguousarray(x.astype(np.float32, copy=False)).reshape(
        B * C, NV, H, W
    )
    in_maps = [{"x": xs[NS * k : NS * (k + 1)]} for k in range(NCORES)]
    res = run_bass_kernel_spmd(_get_nc(), in_maps, core_ids=list(range(NCORES)))
    out = np.concatenate([r["out"] for r in res.results], axis=0)
    return out.reshape(B, C, NV, D, H, W)
